# revision 78
# baseline (speedup 1.0000x reference)
"""AlignmentAttention Trainium2 kernel (8 NeuronCores, pure data parallel over B).

Math: reference computes
    key    = einsum("nbsr,er->nbse", kv, Wk) + bk
    scores = einsum("bte,nbse->nbts", q, key) + mask
    out    = softmax(scores) @ kv
Because softmax is invariant to per-row constants, the bias term q@bk cancels,
and q @ (kv@Wk^T)^T == (q@Wk) @ kv^T.  So we project the *query* once per batch
element (qproj = q@Wk, shared across all N candidates) instead of projecting
keys N times, and drop bk entirely.

Sharding: one batch element b per core (B=8 == n_cores).  Per core:
    qprojT = (q_b @ Wk)^T              64 matmuls   (fp16 operands, f32 psum)
    per candidate n:
        scores  = qproj @ kvT_nb        32 matmuls
        softmax: DVE mask-add + rowmax, ACT fused exp+rowsum -> fp16 attn
        attn^T via fp16 PE transpose (1 cyc/row, fp16 psum)
        out_nb  = attn @ kv_nb          32 matmuls, 1/rowsum fused into the
                  fp16 eviction; host upcasts fp16 -> f32

Perf notes (from perfetto/NTFF analysis; baseline 117.8us -> ~93-96us):
  - engine preambles delay all real work to ~9us; input DMA starts ~8.7us.
    The warmup fp16 identity transposes stream through that dead zone and
    carry the PE p-state ramp (0.65 -> 1.2 -> 2.4 GHz after ~5us of
    *uninterrupted* activity; any idle gap resets it) into qproj.
  - qproj is e-major in 7+1 r-block passes (7 psum banks; the transpose
    psum pool gives up its double buffer) so its matmul stream chases the
    wk/qT chunk arrivals and only one r-block's accumulation serializes
    after the last chunk; qT/wk stream e-ordered across three balanced
    in-order DMA queues (sync/scalar/gpsimd, ~1MB each).
  - kv/kvT pools are 3 deep and per-candidate loads are 2 DMAs each:
    prefetch hides the buffer-reuse serialization that starves scores.
  - per-candidate PE order S0 S1 S2 T0 S3 T1 O0 T2 O1 T3 O2 O3: each fp16
    attn transpose block runs well before its outs need the attnT copy, so
    scores-end -> first-out has no transpose/copy latency (14.9us/cand,
    zero-stall; 64 x 216ns matmuls + 16 transposes).
  - softmax normalization is deferred to the host: the kernel ships fp16
    unnormalized out tiles + packed f32 row sums (one [128,16] DMA), so no
    on-device reciprocal (a DVE reciprocal waiting on the ACT accumulator
    blocks the in-order DVE queue) and evictions are plain copies.
  - deep pools (scorepool 4, attnpool 4, outpool 6) keep ring-reuse
    dependencies off the critical path; PE idle gaps are doubly expensive
    because they also trigger HAM duty-cycle dips (k=8/8 -> 4/8).
  - out is fp16 on the gpsimd queue: halves tail-store bytes and keeps
    DMA issue off the scalar engine (which runs exp + evictions).
"""
import contextlib
import os
import sys

import numpy as np

_TRN_REPO = "/opt/trn_rl_repo"
if _TRN_REPO not in sys.path and os.path.isdir(_TRN_REPO):
    sys.path.insert(0, _TRN_REPO)

# jax on the native neuron backend crashes; the axon PJRT proxy path needs the
# default platform selection.
if os.environ.get("JAX_PLATFORMS") == "cpu":
    os.environ["JAX_PLATFORMS"] = ""

import concourse.bacc as bacc
import concourse.tile as tile
from concourse import mybir
from concourse.bass_utils import run_bass_kernel_spmd

F32 = mybir.dt.float32
F16 = mybir.dt.float16

N_CAND, B, T, S, E, R = 4, 8, 512, 512, 1024, 1024
TT, ST, ET, RT = T // 128, S // 128, E // 128, R // 128

_NC_CACHE = []


def build_nc():
    nc = bacc.Bacc(None, target_bir_lowering=False)
    qT = nc.declare_dram_parameter("qT", [E, T], F16, isOutput=False)
    kv = nc.declare_dram_parameter("kv", [N_CAND, S, R], F16, isOutput=False)
    kvT = nc.declare_dram_parameter("kvT", [N_CAND, R, S], F16, isOutput=False)
    mask = nc.declare_dram_parameter("mask", [T, S], F16, isOutput=False)
    wk = nc.declare_dram_parameter("wk", [E, R], F16, isOutput=False)
    ident = nc.declare_dram_parameter("ident", [128, 128], F16, isOutput=False)
    out = nc.declare_dram_parameter("out", [N_CAND, T, R], F16, isOutput=True)
    # unnormalized-softmax row sums, packed [t_lo, n*TT+ti]; the host divides
    # them out during unshard
    sums = nc.declare_dram_parameter("sums", [128, N_CAND * TT], F32, isOutput=True)

    with contextlib.ExitStack() as ctx:
        tc = ctx.enter_context(tile.TileContext(nc))
        singles = ctx.enter_context(tc.tile_pool(name="singles", bufs=1))
        kvpool = ctx.enter_context(tc.tile_pool(name="kvpool", bufs=3))
        kvtpool = ctx.enter_context(tc.tile_pool(name="kvtpool", bufs=3))
        scorepool = ctx.enter_context(tc.tile_pool(name="scorepool", bufs=4))
        attnpool = ctx.enter_context(tc.tile_pool(name="attnpool", bufs=4))
        attntpool = ctx.enter_context(tc.tile_pool(name="attntpool", bufs=2))
        outpool = ctx.enter_context(tc.tile_pool(name="outpool", bufs=6))
        smalls = ctx.enter_context(tc.tile_pool(name="smalls", bufs=10))
        # psT is a single bank: each transpose psum's copy completes >=1.7us
        # before the next transpose block needs the slot (an out-matmul block
        # sits between them in PE order), so double-buffering buys nothing,
        # and the freed bank gives qproj a 7th accumulator.
        psT = ctx.enter_context(tc.tile_pool(name="psT", bufs=1, space="PSUM"))
        psmm = ctx.enter_context(tc.tile_pool(name="psmm", bufs=7, space="PSUM"))

        wk_sb = singles.tile([128, ET, R], F16)
        qT_sb = singles.tile([128, ET, T], F16)
        ident16 = singles.tile([128, 128], F16)

        # Input staging across three balanced in-order DMA queues (~1MB
        # each), issued so chunks land in e-order for the e-major qproj
        # sweep: qT pairs on sync, even wk chunks on scalar, odd wk chunks
        # on gpsimd.  The identity is shipped from the host (a gpsimd
        # memset/affine_select build would delay that queue's first wk
        # transfer by ~2.5us, and the PE warmups depend on the identity).
        nc.sync.dma_start(out=ident16, in_=ident[:, :])
        for h in range(4):
            nc.gpsimd.dma_start(out=wk_sb[:, 2 * h + 1, :],
                                in_=wk[(2 * h + 1) * 128:(2 * h + 2) * 128, :])
        nc.sync.dma_start(
            out=qT_sb[:, 0:2, :],
            in_=qT[0:256, :].rearrange("(eh p) t -> p eh t", p=128))
        nc.scalar.dma_start(out=wk_sb[:, 0, :], in_=wk[0:128, :])
        # e2/e3 as single chunks: they are the first arrivals the sweep can
        # stall on, and smaller DMAs become PE-visible sooner
        nc.sync.dma_start(out=qT_sb[:, 2, :], in_=qT[256:384, :])
        nc.scalar.dma_start(out=wk_sb[:, 2, :], in_=wk[256:384, :])
        nc.sync.dma_start(out=qT_sb[:, 3, :], in_=qT[384:512, :])
        nc.scalar.dma_start(out=wk_sb[:, 4, :], in_=wk[512:640, :])
        nc.sync.dma_start(
            out=qT_sb[:, 4:6, :],
            in_=qT[512:768, :].rearrange("(eh p) t -> p eh t", p=128))
        nc.scalar.dma_start(out=wk_sb[:, 6, :], in_=wk[768:896, :])
        nc.sync.dma_start(
            out=qT_sb[:, 6:8, :],
            in_=qT[768:1024, :].rearrange("(eh p) t -> p eh t", p=128))

        # Dead-zone warmup: these transposes are free (PE would otherwise
        # idle until the first chunks land) and carry the PE p-state ramp
        # (full clock needs ~5us of *uninterrupted* activity; any idle gap
        # resets it) into qproj.
        # 20 warmup transposes, gated on the identity DMA (~10us): ends right
        # at first-chunk-consumable (~13us), so the clock ramp completes
        # ~1.4us into qproj — deliberately NOT earlier.  A fully-ramped PE
        # outruns the 3-queue chunk feed, and the resulting starvation gap
        # triggers a HAM duty dip that costs more than the ramp tax.
        wp = psT.tile([128, 512], F16, tag="pT", name="wp")
        for k in range(20):
            nc.tensor.transpose(wp[:, (k % 4) * 128:(k % 4 + 1) * 128],
                                ident16, ident16)
        mask_sb = singles.tile([128, TT, S], F16)
        qprojT = singles.tile([128, RT, T], F16)
        sums_sb = singles.tile([128, N_CAND * TT], F32)

        # qprojT[r, t] = sum_e wk[e, r] * qT[e, t]
        # e-major in two passes of 7+1 r-blocks (7 live psum banks): the mm
        # stream chases the wk/qT chunk arrivals, and only one r-block's
        # accumulation remains serialized after the last chunk lands.
        for r0, r1 in ((0, 7), (7, RT)):
            ps = {r: psmm.tile([128, T], F32, name=f"qp{r}", tag="p")
                  for r in range(r0, r1)}
            for e in range(ET):
                for r in range(r0, r1):
                    nc.tensor.matmul(ps[r], wk_sb[:, e, r * 128:(r + 1) * 128],
                                     qT_sb[:, e, :],
                                     start=(e == 0), stop=(e == ET - 1))
            for r in range(r0, r1):
                nc.scalar.copy(qprojT[:, r, :], ps[r])

        for n in range(N_CAND):
            kvT_sb = kvtpool.tile([128, RT, S], F16)
            for h in range(2):
                nc.sync.dma_start(
                    out=kvT_sb[:, 4 * h:4 * h + 4, :],
                    in_=kvT[n, 512 * h:512 * (h + 1), :].rearrange(
                        "(rh p) s -> p rh s", p=128))
            kv_sb = kvpool.tile([128, ST, R], F16)
            for h in range(2):
                nc.sync.dma_start(
                    out=kv_sb[:, 2 * h:2 * h + 2, :],
                    in_=kv[n, 256 * h:256 * (h + 1), :].rearrange(
                        "(sh p) r -> p sh r", p=128))
            if n == 0:
                # mask is needed only at the first softmax; keep it behind
                # the first candidate's kv loads
                nc.sync.dma_start(out=mask_sb,
                                  in_=mask.rearrange("(th p) s -> p th s", p=128))

            # Software-pipelined emission.  PE program order per candidate is
            #   S0 S1 S2 [T0] S3 [T1] O0 [T2] O1 [T3] O2 O3
            # so each transpose block T(ti) runs well before its outs O(ti)
            # need the attnT copy — the scores-end -> first-out critical path
            # has no transpose/copy latency on it.  Engine side streams:
            #   DVE: add/red per ti interleaved with the attnT copies
            #   ACT: exp per ti, then the out evictions
            score_ps = [None] * TT
            attns = [None] * TT
            attnT = attntpool.tile([128, ST, T], F16)

            def scores_mms(ti):
                p = psmm.tile([128, S], F32, name="p")
                for ri in range(RT):
                    nc.tensor.matmul(p, qprojT[:, ri, ti * 128:(ti + 1) * 128],
                                     kvT_sb[:, ri, :],
                                     start=(ri == 0), stop=(ri == RT - 1))
                score_ps[ti] = p

            def softmax(ti):
                # unnormalized: attn_u = exp(scores + mask - rowmax) in fp16;
                # 1/rowsum is deferred to the out-matmul eviction
                scoresN = scorepool.tile([128, S], F32, name="scoresN")
                negmax = smalls.tile([128, 1], F32, name="negmax")
                nc.vector.tensor_add(scoresN, score_ps[ti], mask_sb[:, ti, :])
                nc.vector.tensor_reduce(negmax, scoresN, axis=mybir.AxisListType.X,
                                        op=mybir.AluOpType.max, negate=True)
                attn = attnpool.tile([128, S], F16, name="attn")
                nc.scalar.activation(attn, scoresN, mybir.ActivationFunctionType.Exp,
                                     bias=negmax, scale=1.0,
                                     accum_out=sums_sb[:, n * TT + ti:n * TT + ti + 1])
                attns[ti] = attn

            def transpose_copy(ti):
                pT = psT.tile([128, 512], F16, name="pT", tag="pT")
                for si in range(ST):
                    nc.tensor.transpose(pT[:, si * 128:(si + 1) * 128],
                                        attns[ti][:, si * 128:(si + 1) * 128],
                                        ident16)
                nc.vector.tensor_copy(
                    attnT[:, 0:ST, ti * 128:(ti + 1) * 128],
                    pT.rearrange("p (k j) -> p k j", k=ST))


            def out_mms(ti):
                # out_u[t, r] = sum_s attn_u[t, s] kv[s, r]; the softmax
                # normalization (1/rowsum) happens host-side with the shipped
                # sums, so the eviction is a plain fp16 copy.
                for rh in range(2):
                    p = psmm.tile([128, 512], F32, name="p")
                    for si in range(ST):
                        nc.tensor.matmul(p, attnT[:, si, ti * 128:(ti + 1) * 128],
                                         kv_sb[:, si, rh * 512:(rh + 1) * 512],
                                         start=(si == 0), stop=(si == ST - 1))
                    o = outpool.tile([128, 512], F16, name="o")
                    if n == N_CAND - 1 and ti == TT - 1 and rh == 1:
                        # very last tile: evict in two halves on DVE + ACT in
                        # parallel and store the halves on separate queues —
                        # shortens the last-matmul -> final-store chain
                        nc.vector.tensor_copy(o[:, 0:256], p[:, 0:256])
                        nc.scalar.copy(o[:, 256:512], p[:, 256:512])
                        nc.sync.dma_start(
                            out=out[n, ti * 128:(ti + 1) * 128, 512:768],
                            in_=o[:, 0:256])
                        nc.gpsimd.dma_start(
                            out=out[n, ti * 128:(ti + 1) * 128, 768:1024],
                            in_=o[:, 256:512])
                    else:
                        nc.scalar.copy(o, p)
                        nc.gpsimd.dma_start(
                            out=out[n, ti * 128:(ti + 1) * 128,
                                    rh * 512:(rh + 1) * 512],
                            in_=o)

            scores_mms(0)
            softmax(0)
            scores_mms(1)
            softmax(1)
            scores_mms(2)
            softmax(2)
            transpose_copy(0)
            scores_mms(3)
            softmax(3)
            transpose_copy(1)
            out_mms(0)
            transpose_copy(2)
            out_mms(1)
            transpose_copy(3)
            out_mms(2)
            out_mms(3)

        # sync queue: idle by now, and keeps this off the gpsimd drain chain
        # that flushes the final out stores
        nc.sync.dma_start(out=sums[:, :], in_=sums_sb)

    nc.compile()
    return nc


def make_in_maps(query, key_value_states, attention_mask, Wk):
    in_maps = []
    for b in range(B):
        in_maps.append({
            "qT": np.ascontiguousarray(query[0, b].T).astype(np.float16),
            "kv": np.ascontiguousarray(key_value_states[:, b]).astype(np.float16),
            "kvT": np.ascontiguousarray(
                key_value_states[:, b].transpose(0, 2, 1)).astype(np.float16),
            "mask": np.ascontiguousarray(attention_mask[0, b]).astype(np.float16),
            "ident": np.eye(128, dtype=np.float16),
            "wk": np.ascontiguousarray(Wk).astype(np.float16),
        })
    return in_maps


def kernel(query, key_value_states, attention_mask, Wk, bk):
    query = np.asarray(query, dtype=np.float32)
    key_value_states = np.asarray(key_value_states, dtype=np.float32)
    attention_mask = np.asarray(attention_mask, dtype=np.float32)
    Wk = np.asarray(Wk, dtype=np.float32)
    del bk  # cancels inside the softmax (constant along the softmax axis)

    if not _NC_CACHE:
        _NC_CACHE.append(build_nc())
    nc = _NC_CACHE[0]

    in_maps = make_in_maps(query, key_value_states, attention_mask, Wk)
    res = run_bass_kernel_spmd(nc, in_maps, core_ids=list(range(B)))

    out = np.empty((N_CAND, B, T, R), dtype=np.float32)
    for b in range(B):
        # sums_sb is [t_lo, n*TT+ti]; rowsum(n, ti*128+t_lo) = sums[t_lo, n*TT+ti]
        s = res.results[b]["sums"].astype(np.float32)
        rowsum = s.reshape(128, N_CAND, TT).transpose(1, 2, 0).reshape(N_CAND, T)
        out[:, b] = res.results[b]["out"].astype(np.float32) / rowsum[:, :, None]
    return out


# revision 79
# speedup vs baseline: 1.0110x; 1.0110x over previous
"""AlignmentAttention Trainium2 kernel (8 NeuronCores, pure data parallel over B).

Math: reference computes
    key    = einsum("nbsr,er->nbse", kv, Wk) + bk
    scores = einsum("bte,nbse->nbts", q, key) + mask
    out    = softmax(scores) @ kv
Because softmax is invariant to per-row constants, the bias term q@bk cancels,
and q @ (kv@Wk^T)^T == (q@Wk) @ kv^T.  So we project the *query* once per batch
element (qproj = q@Wk, shared across all N candidates) instead of projecting
keys N times, and drop bk entirely.

Sharding: one batch element b per core (B=8 == n_cores).  Per core:
    qprojT = (q_b @ Wk)^T              64 matmuls   (fp16 operands, f32 psum)
    per candidate n:
        scores  = qproj @ kvT_nb        32 matmuls
        softmax: DVE mask-add + rowmax, ACT fused exp+rowsum -> fp16 attn
        attn^T via fp16 PE transpose (1 cyc/row, fp16 psum)
        out_nb  = attn @ kv_nb          32 matmuls, 1/rowsum fused into the
                  fp16 eviction; host upcasts fp16 -> f32

Perf notes (from perfetto/NTFF analysis; baseline 117.8us -> ~93-96us):
  - engine preambles delay all real work to ~9us; input DMA starts ~8.7us.
    The warmup fp16 identity transposes stream through that dead zone and
    carry the PE p-state ramp (0.65 -> 1.2 -> 2.4 GHz after ~5us of
    *uninterrupted* activity; any idle gap resets it) into qproj.
  - qproj is e-major in 7+1 r-block passes (7 psum banks; the transpose
    psum pool gives up its double buffer) so its matmul stream chases the
    wk/qT chunk arrivals and only one r-block's accumulation serializes
    after the last chunk; qT/wk stream e-ordered across three balanced
    in-order DMA queues (sync/scalar/gpsimd, ~1MB each).
  - kv/kvT pools are 3 deep and per-candidate loads are 2 DMAs each:
    prefetch hides the buffer-reuse serialization that starves scores.
  - per-candidate PE order S0 S1 S2 T0 S3 T1 O0 T2 O1 T3 O2 O3: each fp16
    attn transpose block runs well before its outs need the attnT copy, so
    scores-end -> first-out has no transpose/copy latency (14.9us/cand,
    zero-stall; 64 x 216ns matmuls + 16 transposes).
  - softmax normalization is deferred to the host: the kernel ships fp16
    unnormalized out tiles + packed f32 row sums (one [128,16] DMA), so no
    on-device reciprocal (a DVE reciprocal waiting on the ACT accumulator
    blocks the in-order DVE queue) and evictions are plain copies.
  - deep pools (scorepool 4, attnpool 4, outpool 6) keep ring-reuse
    dependencies off the critical path; PE idle gaps are doubly expensive
    because they also trigger HAM duty-cycle dips (k=8/8 -> 4/8).
  - out is fp16 on the gpsimd queue: halves tail-store bytes and keeps
    DMA issue off the scalar engine (which runs exp + evictions).
"""
import contextlib
import os
import sys

import numpy as np

_TRN_REPO = "/opt/trn_rl_repo"
if _TRN_REPO not in sys.path and os.path.isdir(_TRN_REPO):
    sys.path.insert(0, _TRN_REPO)

# jax on the native neuron backend crashes; the axon PJRT proxy path needs the
# default platform selection.
if os.environ.get("JAX_PLATFORMS") == "cpu":
    os.environ["JAX_PLATFORMS"] = ""

import concourse.bacc as bacc
import concourse.tile as tile
from concourse import mybir
from concourse.bass_utils import run_bass_kernel_spmd

F32 = mybir.dt.float32
F16 = mybir.dt.float16

N_CAND, B, T, S, E, R = 4, 8, 512, 512, 1024, 1024
TT, ST, ET, RT = T // 128, S // 128, E // 128, R // 128

_NC_CACHE = []


def build_nc():
    nc = bacc.Bacc(None, target_bir_lowering=False)
    qT = nc.declare_dram_parameter("qT", [E, T], F16, isOutput=False)
    kv = nc.declare_dram_parameter("kv", [N_CAND, S, R], F16, isOutput=False)
    kvT = nc.declare_dram_parameter("kvT", [N_CAND, R, S], F16, isOutput=False)
    mask = nc.declare_dram_parameter("mask", [T, S], F16, isOutput=False)
    wk = nc.declare_dram_parameter("wk", [E, R], F16, isOutput=False)
    ident = nc.declare_dram_parameter("ident", [128, 128], F16, isOutput=False)
    out = nc.declare_dram_parameter("out", [N_CAND, T, R], F16, isOutput=True)
    # unnormalized-softmax row sums, packed [t_lo, n*TT+ti]; the host divides
    # them out during unshard
    sums = nc.declare_dram_parameter("sums", [128, N_CAND * TT], F32, isOutput=True)

    with contextlib.ExitStack() as ctx:
        tc = ctx.enter_context(tile.TileContext(nc))
        singles = ctx.enter_context(tc.tile_pool(name="singles", bufs=1))
        kvpool = ctx.enter_context(tc.tile_pool(name="kvpool", bufs=3))
        kvtpool = ctx.enter_context(tc.tile_pool(name="kvtpool", bufs=3))
        scorepool = ctx.enter_context(tc.tile_pool(name="scorepool", bufs=4))
        attnpool = ctx.enter_context(tc.tile_pool(name="attnpool", bufs=4))
        attntpool = ctx.enter_context(tc.tile_pool(name="attntpool", bufs=2))
        outpool = ctx.enter_context(tc.tile_pool(name="outpool", bufs=6))
        smalls = ctx.enter_context(tc.tile_pool(name="smalls", bufs=10))
        # psT is a single bank: each transpose psum's copy completes >=1.7us
        # before the next transpose block needs the slot (an out-matmul block
        # sits between them in PE order), so double-buffering buys nothing,
        # and the freed bank gives qproj a 7th accumulator.
        psT = ctx.enter_context(tc.tile_pool(name="psT", bufs=1, space="PSUM"))
        psmm = ctx.enter_context(tc.tile_pool(name="psmm", bufs=7, space="PSUM"))

        wk_sb = singles.tile([128, ET, R], F16)
        qT_sb = singles.tile([128, ET, T], F16)
        ident16 = singles.tile([128, 128], F16)

        # Input staging across three balanced in-order DMA queues (~1MB
        # each), issued so chunks land in e-order for the e-major qproj
        # sweep: qT pairs on sync, even wk chunks on scalar, odd wk chunks
        # on gpsimd.  The identity is shipped from the host (a gpsimd
        # memset/affine_select build would delay that queue's first wk
        # transfer by ~2.5us, and the PE warmups depend on the identity).
        nc.sync.dma_start(out=ident16, in_=ident[:, :])
        for h in range(4):
            nc.gpsimd.dma_start(out=wk_sb[:, 2 * h + 1, :],
                                in_=wk[(2 * h + 1) * 128:(2 * h + 2) * 128, :])
        nc.sync.dma_start(
            out=qT_sb[:, 0:2, :],
            in_=qT[0:256, :].rearrange("(eh p) t -> p eh t", p=128))
        nc.scalar.dma_start(out=wk_sb[:, 0, :], in_=wk[0:128, :])
        # e2/e3 as single chunks: they are the first arrivals the sweep can
        # stall on, and smaller DMAs become PE-visible sooner
        nc.sync.dma_start(out=qT_sb[:, 2, :], in_=qT[256:384, :])
        nc.scalar.dma_start(out=wk_sb[:, 2, :], in_=wk[256:384, :])
        nc.sync.dma_start(out=qT_sb[:, 3, :], in_=qT[384:512, :])
        nc.scalar.dma_start(out=wk_sb[:, 4, :], in_=wk[512:640, :])
        nc.sync.dma_start(
            out=qT_sb[:, 4:6, :],
            in_=qT[512:768, :].rearrange("(eh p) t -> p eh t", p=128))
        nc.scalar.dma_start(out=wk_sb[:, 6, :], in_=wk[768:896, :])
        nc.sync.dma_start(
            out=qT_sb[:, 6:8, :],
            in_=qT[768:1024, :].rearrange("(eh p) t -> p eh t", p=128))

        # Dead-zone warmup: these transposes are free (PE would otherwise
        # idle until the first chunks land) and carry the PE p-state ramp
        # (full clock needs ~5us of *uninterrupted* activity; any idle gap
        # resets it) into qproj.
        # Warmups: plain matmuls on a DVE-memset zero tile need no DMA'd
        # input, so they start at the PE preamble end (~7.5us).  The count
        # matters: they must stream until first-chunk-consumable (~13us) so
        # (a) the ~5us clock ramp completes right at the handoff (no
        # mid-clock qproj tax) and (b) ~4 chunks are already buffered —
        # ending early hands a fully-ramped PE an empty feed, and the
        # starvation gap triggers a HAM duty dip (measured: +3us).
        zwarm = singles.tile([128, 128], F16)
        nc.vector.memset(zwarm, 0.0)
        wq = psmm.tile([128, 512], F32, tag="p", name="wq")
        for k in range(54):
            nc.tensor.matmul(wq[:, (k % 4) * 128:(k % 4 + 1) * 128],
                             zwarm, zwarm, start=True, stop=True)
        mask_sb = singles.tile([128, TT, S], F16)
        qprojT = singles.tile([128, RT, T], F16)
        sums_sb = singles.tile([128, N_CAND * TT], F32)

        # qprojT[r, t] = sum_e wk[e, r] * qT[e, t]
        # e-major in two passes of 7+1 r-blocks (7 live psum banks): the mm
        # stream chases the wk/qT chunk arrivals, and only one r-block's
        # accumulation remains serialized after the last chunk lands.
        for r0, r1 in ((0, 7), (7, RT)):
            ps = {r: psmm.tile([128, T], F32, name=f"qp{r}", tag="p")
                  for r in range(r0, r1)}
            for e in range(ET):
                for r in range(r0, r1):
                    nc.tensor.matmul(ps[r], wk_sb[:, e, r * 128:(r + 1) * 128],
                                     qT_sb[:, e, :],
                                     start=(e == 0), stop=(e == ET - 1))
            for r in range(r0, r1):
                nc.scalar.copy(qprojT[:, r, :], ps[r])

        for n in range(N_CAND):
            kvT_sb = kvtpool.tile([128, RT, S], F16)
            for h in range(2):
                nc.sync.dma_start(
                    out=kvT_sb[:, 4 * h:4 * h + 4, :],
                    in_=kvT[n, 512 * h:512 * (h + 1), :].rearrange(
                        "(rh p) s -> p rh s", p=128))
            kv_sb = kvpool.tile([128, ST, R], F16)
            for h in range(2):
                nc.sync.dma_start(
                    out=kv_sb[:, 2 * h:2 * h + 2, :],
                    in_=kv[n, 256 * h:256 * (h + 1), :].rearrange(
                        "(sh p) r -> p sh r", p=128))
            if n == 0:
                # mask is needed only at the first softmax; keep it behind
                # the first candidate's kv loads
                nc.sync.dma_start(out=mask_sb,
                                  in_=mask.rearrange("(th p) s -> p th s", p=128))

            # Software-pipelined emission.  PE program order per candidate is
            #   S0 S1 S2 [T0] S3 [T1] O0 [T2] O1 [T3] O2 O3
            # so each transpose block T(ti) runs well before its outs O(ti)
            # need the attnT copy — the scores-end -> first-out critical path
            # has no transpose/copy latency on it.  Engine side streams:
            #   DVE: add/red per ti interleaved with the attnT copies
            #   ACT: exp per ti, then the out evictions
            score_ps = [None] * TT
            attns = [None] * TT
            attnT = attntpool.tile([128, ST, T], F16)

            def scores_mms(ti):
                p = psmm.tile([128, S], F32, name="p")
                for ri in range(RT):
                    nc.tensor.matmul(p, qprojT[:, ri, ti * 128:(ti + 1) * 128],
                                     kvT_sb[:, ri, :],
                                     start=(ri == 0), stop=(ri == RT - 1))
                score_ps[ti] = p

            def softmax(ti):
                # unnormalized: attn_u = exp(scores + mask - rowmax) in fp16;
                # 1/rowsum is deferred to the out-matmul eviction
                scoresN = scorepool.tile([128, S], F32, name="scoresN")
                negmax = smalls.tile([128, 1], F32, name="negmax")
                nc.vector.tensor_add(scoresN, score_ps[ti], mask_sb[:, ti, :])
                nc.vector.tensor_reduce(negmax, scoresN, axis=mybir.AxisListType.X,
                                        op=mybir.AluOpType.max, negate=True)
                attn = attnpool.tile([128, S], F16, name="attn")
                nc.scalar.activation(attn, scoresN, mybir.ActivationFunctionType.Exp,
                                     bias=negmax, scale=1.0,
                                     accum_out=sums_sb[:, n * TT + ti:n * TT + ti + 1])
                attns[ti] = attn

            def transpose_copy(ti):
                pT = psT.tile([128, 512], F16, name="pT", tag="pT")
                for si in range(ST):
                    nc.tensor.transpose(pT[:, si * 128:(si + 1) * 128],
                                        attns[ti][:, si * 128:(si + 1) * 128],
                                        ident16)
                nc.vector.tensor_copy(
                    attnT[:, 0:ST, ti * 128:(ti + 1) * 128],
                    pT.rearrange("p (k j) -> p k j", k=ST))


            def out_mms(ti):
                # out_u[t, r] = sum_s attn_u[t, s] kv[s, r]; the softmax
                # normalization (1/rowsum) happens host-side with the shipped
                # sums, so the eviction is a plain fp16 copy.
                for rh in range(2):
                    p = psmm.tile([128, 512], F32, name="p")
                    for si in range(ST):
                        nc.tensor.matmul(p, attnT[:, si, ti * 128:(ti + 1) * 128],
                                         kv_sb[:, si, rh * 512:(rh + 1) * 512],
                                         start=(si == 0), stop=(si == ST - 1))
                    o = outpool.tile([128, 512], F16, name="o")
                    if n == N_CAND - 1 and ti == TT - 1 and rh == 1:
                        # very last tile: evict in two halves on DVE + ACT in
                        # parallel and store the halves on separate queues —
                        # shortens the last-matmul -> final-store chain
                        nc.vector.tensor_copy(o[:, 0:256], p[:, 0:256])
                        nc.scalar.copy(o[:, 256:512], p[:, 256:512])
                        nc.sync.dma_start(
                            out=out[n, ti * 128:(ti + 1) * 128, 512:768],
                            in_=o[:, 0:256])
                        nc.gpsimd.dma_start(
                            out=out[n, ti * 128:(ti + 1) * 128, 768:1024],
                            in_=o[:, 256:512])
                    else:
                        nc.scalar.copy(o, p)
                        nc.gpsimd.dma_start(
                            out=out[n, ti * 128:(ti + 1) * 128,
                                    rh * 512:(rh + 1) * 512],
                            in_=o)

            scores_mms(0)
            softmax(0)
            scores_mms(1)
            softmax(1)
            scores_mms(2)
            softmax(2)
            transpose_copy(0)
            scores_mms(3)
            softmax(3)
            transpose_copy(1)
            out_mms(0)
            transpose_copy(2)
            out_mms(1)
            transpose_copy(3)
            out_mms(2)
            out_mms(3)

        # sync queue: idle by now, and keeps this off the gpsimd drain chain
        # that flushes the final out stores
        nc.sync.dma_start(out=sums[:, :], in_=sums_sb)

    nc.compile()
    return nc


def make_in_maps(query, key_value_states, attention_mask, Wk):
    in_maps = []
    for b in range(B):
        in_maps.append({
            "qT": np.ascontiguousarray(query[0, b].T).astype(np.float16),
            "kv": np.ascontiguousarray(key_value_states[:, b]).astype(np.float16),
            "kvT": np.ascontiguousarray(
                key_value_states[:, b].transpose(0, 2, 1)).astype(np.float16),
            "mask": np.ascontiguousarray(attention_mask[0, b]).astype(np.float16),
            "ident": np.eye(128, dtype=np.float16),
            "wk": np.ascontiguousarray(Wk).astype(np.float16),
        })
    return in_maps


def kernel(query, key_value_states, attention_mask, Wk, bk):
    query = np.asarray(query, dtype=np.float32)
    key_value_states = np.asarray(key_value_states, dtype=np.float32)
    attention_mask = np.asarray(attention_mask, dtype=np.float32)
    Wk = np.asarray(Wk, dtype=np.float32)
    del bk  # cancels inside the softmax (constant along the softmax axis)

    if not _NC_CACHE:
        _NC_CACHE.append(build_nc())
    nc = _NC_CACHE[0]

    in_maps = make_in_maps(query, key_value_states, attention_mask, Wk)
    res = run_bass_kernel_spmd(nc, in_maps, core_ids=list(range(B)))

    out = np.empty((N_CAND, B, T, R), dtype=np.float32)
    for b in range(B):
        # sums_sb is [t_lo, n*TT+ti]; rowsum(n, ti*128+t_lo) = sums[t_lo, n*TT+ti]
        s = res.results[b]["sums"].astype(np.float32)
        rowsum = s.reshape(128, N_CAND, TT).transpose(1, 2, 0).reshape(N_CAND, T)
        out[:, b] = res.results[b]["out"].astype(np.float32) / rowsum[:, :, None]
    return out
